# revision 25
# baseline (speedup 1.0000x reference)
"""NetsVocab per-word MLP kernel for 8 Trainium2 NeuronCores.

Math (per active word w of 16, per box b of 8192):
    h1 = relu(x @ W1[w] + b1[w])        # [B,4096] @ [4096,64]
    h2 = relu(h1 @ W2[w] + b2[w])       # [B,64] @ [64,32]
    l  = h2 @ W3[w] + b3[w]             # [B,32] @ [32]
    out[b] = prod_w sigmoid(l[w,b])

Strategy: data-parallel over boxes (1024 per core), the 16 active words'
weights gathered on host and replicated to all cores; no collectives.
Layer 1 dominates (8.6 GFLOP/core): 512 bf16 PE matmuls of
[128k x 128m x 512n] ~ 109 us at the warm 2.4 GHz roofline. Layers 2/3
are tiny block-diagonal matmuls (2 words per tile) zero-padded to 128
lhsT columns so their weight loads stay FWL-eligible.

Schedule (measured-latency driven):
  - 0-6.5us is fixed framework preamble; DMA triggers can't start
    earlier, and the first wave lands ~4us after issue (pipe-fill +
    HBM receipt under 8-core load) -> first k-tile ~10.5-11us.
  - A 40-matmul N=128 warmup chain keeps the PE busy without gaps from
    ~7.1us until the first data wave, so the HAM clock-gate unthrottles
    (~3.4us sustained busy) right as real work starts. Any idle gap
    there restarts the HAM window and halves the clock for ~10+ us.
  - n-major passes bound DMA demand: pass 1 = {m0..m5} x n0 consumes
    xT-n0 at 100 GB/s + w1a (m0..m5 columns) at 150 GB/s - comfortably
    inside the ~350 GB/s/core wire even with 8 cores loading at once.
    Pass 2 = {m0..m5} x n1 and pass 3 = {m6,m7} x {n0,n1} consume
    already-resident data, so only pass 1 is arrival-paced.
  - Epilogues run as a 3-stage pipeline ticked between k-matmuls so the
    in-order PE never waits on ACT products. The 16-sigmoid product
    accumulates into a [2, BC] tile; partition 1 is combined via one
    SBUF->SBUF DMA bounce (engines address base partitions {0,32,64,96}
    only), m7's words go to lhsT cols 0/32 so their logits are readable
    and multiply into the output directly.
  - (7,1) runs as two 256-box k-loops: half a's whole epilogue chain +
    out-DMA hide under half b's k-loop, so only half b's ~3us chain and
    one 1 KiB out-DMA receipt remain after the final L1 matmul.

Layouts (per core):
    xT  [128, 2, KT, 512] bf16  [p, nhalf, k, b]
    w1a [128, KT, 768] bf16     m0..m5 columns (wh 0..767), k-tile major
    w1b [128, KT, 256] bf16     m6,m7 columns
    w2  [128, 8, 128] bf16      per m-tile block-diag, zero-padded cols
    w3  [64, 8, 128] bf16       block-diag cols 0/1 (m=7: cols 0/32)
    bc  [128, 24] f32           b1 | b2 | b3 packed, out [1, 1024] f32
"""

import os

import numpy as np
import ml_dtypes

import concourse.bass as bass
import concourse.tile as tile
from concourse import bacc
from concourse import mybir
from concourse.bass import ts
from concourse.bass_utils import run_bass_kernel_spmd

BF16 = mybir.dt.bfloat16
F32 = mybir.dt.float32
AF = mybir.ActivationFunctionType

N_CORES = 8
B = 8192            # total boxes
BC = B // N_CORES   # boxes per core (1024)
F = 4096            # features
NW = 16             # active words
H1 = 64
H2 = 32
KT = F // 128       # 32 k-tiles
MT = NW * H1 // 128  # 8 m-tiles (wh = w*64+h, 2 words per tile)

LAST_RESULTS = None  # BassKernelResults of the most recent run (for test.py)


def build_nc():
    nc = bacc.Bacc("TRN2", target_bir_lowering=False, debug=False)

    xT_d = nc.dram_tensor("xT", [128, 2, KT, 512], BF16, kind="ExternalInput")
    w1a_d = nc.dram_tensor("w1a", [128, KT, 768], BF16, kind="ExternalInput")
    w1b_d = nc.dram_tensor("w1b", [128, KT, 256], BF16, kind="ExternalInput")
    w2_d = nc.dram_tensor("w2", [128, MT, 128], BF16, kind="ExternalInput")
    w3_d = nc.dram_tensor("w3", [128, MT, 128], BF16, kind="ExternalInput")
    # b1/b2/b3 packed into one [128, 3*MT] f32 tensor (cols 0:8 b1,
    # 8:16 b2 (rows 0:64), 16:24 b3 (rows 0:33)): one 96 B/partition DMA
    # instead of three 32 B/partition ones.
    bc_d = nc.dram_tensor("bc", [128, 3 * MT], F32, kind="ExternalInput")
    out_d = nc.dram_tensor("out", [1, BC], F32, kind="ExternalOutput")
    wsink_d = nc.dram_tensor("wsink", [1, 8], F32)

    with tile.TileContext(nc) as tc:
        with (
            tc.tile_pool(name="big", bufs=1) as big,
            tc.tile_pool(name="smalls", bufs=1) as smalls,
            tc.tile_pool(name="h1p", bufs=8) as h1p,
            tc.tile_pool(name="h2p", bufs=8) as h2p,
            tc.tile_pool(name="sigp", bufs=4) as sigp,
            tc.tile_pool(name="prodp", bufs=1) as prodp,
            tc.tile_pool(name="accp", bufs=6, space="PSUM") as accp,
            tc.tile_pool(name="ps2p", bufs=1, space="PSUM") as ps2p,
            tc.tile_pool(name="ps3p", bufs=1, space="PSUM") as ps3p,
        ):
            w2_sb = smalls.tile([128, MT, 128], BF16, tag="w2", name="w2_sb")
            w3_sb = smalls.tile([128, MT, 128], BF16, tag="w3", name="w3_sb")
            bc_sb = smalls.tile([128, 3 * MT], F32, tag="bc", name="bc_sb")

            xT_sb = big.tile([128, 2, KT, 512], BF16, tag="xT", name="xT_sb")
            w1a_sb = big.tile([128, KT, 768], BF16, tag="w1a", name="w1a_sb")
            w1b_sb = big.tile([128, KT, 256], BF16, tag="w1b", name="w1b_sb")

            # Pass-1-critical streams with SMALL first waves (~4us pipe
            # fill gates the first k-tile regardless of size). Each
            # dma_start costs ~0.6-0.9us descriptor-gen on its issuing
            # engine and completion sems are an 8-lane pool (trigger N+8
            # blocks on N completing), so triggers are few. xT rides the
            # scalar HWDGE queue, w1 + smalls the sync queue. Data not
            # needed during pass 1 (xT-n1, w1b, smalls) issues after the
            # pass-1-critical waves and lands during pass 1/2's slack.
            # Wave completion sems lag the data by ~3us (HBM write-receipt
            # under 8-core load), so sub-2kt waves stall the k-loop on sem
            # latency rather than wire speed: start at 2kt. A wave's first
            # k-tile is only usable at its LAST byte + that lag, so the
            # front runs 2kt waves matched to the 1.28us/ktile consume
            # rate, growing to 4/8kt once the k-deadline slack exceeds the
            # ~3us arrival jitter. An all-2kt train is WORSE (tried: the
            # per-wave receipt bubbles pushed the tail waves ~4us late and
            # re-throttled the HAM mid-pass) - keep the tail chunky.
            waves = [(0, 2), (2, 2), (4, 2), (6, 2), (8, 4), (12, 4),
                     (16, 8), (24, 8)]
            for k0, nk in waves:
                nc.sync.dma_start(
                    out=w1a_sb[:, k0:k0 + nk, :], in_=w1a_d[:, k0:k0 + nk, :]
                )
                nc.scalar.dma_start(
                    out=xT_sb[:, 0, k0:k0 + nk, :],
                    in_=xT_d[:, 0, k0:k0 + nk, :],
                )
            for c in range(2):
                nc.scalar.dma_start(
                    out=xT_sb[:, 1, ts(c, 16), :], in_=xT_d[:, 1, ts(c, 16), :]
                )
            nc.sync.dma_start(out=w2_sb, in_=w2_d[:])
            nc.sync.dma_start(out=w3_sb, in_=w3_d[:])
            nc.sync.dma_start(out=bc_sb, in_=bc_d[:])
            nc.sync.dma_start(out=w1b_sb, in_=w1b_d[:])

            # Warm up the PE's HAM clock gate during the initial DMA wait.
            # N=128 matmuls on a small memset tile: the chain starts as
            # soon as the ~170ns memset lands (~7.1us, right after the
            # engine preamble) and covers the PE until the first data wave
            # lands (~11us). The HAM unthrottles after ~3.4us of sustained
            # busy (~10.6us), so the real k-loop starts warm.
            warm_src = smalls.tile([128, 128], BF16, tag="warm", name="warm_src")
            nc.vector.memset(warm_src, 0.0)
            # Dummy sigmoid right after the memset: forces the ACT engine's
            # Sigmoid table load (1.28us) to happen here, during the DMA
            # wait, instead of at the first real sigmoid mid-kernel where
            # the PE is blocked on ACT products (measured 1.3us stall at
            # the pass-1->2 boundary without this).
            sig_dummy = smalls.tile([1, 4], F32, tag="sigd", name="sig_dummy")
            nc.scalar.activation(
                sig_dummy, warm_src[0:1, 0:4], AF.Sigmoid
            )
            warm_ps = ps2p.tile([128, 128], F32, tag="ps2", name="warm_ps")
            NWARM = 48
            for wi in range(NWARM):
                nc.tensor.matmul(
                    warm_ps, warm_src, warm_src,
                    start=(wi == 0), stop=(wi == NWARM - 1),
                )
            # Sink the warmup + dummy-sigmoid results to scratch DRAM so
            # DCE can't drop them (issued late on sync: no queue in a
            # hurry; both feed one tiny DMA).
            wsink = smalls.tile([1, 8], F32, tag="wsink", name="wsink")
            nc.vector.tensor_copy(wsink[:, 0:4], warm_ps[0:1, 0:4])
            nc.vector.tensor_copy(wsink[:, 4:8], sig_dummy)
            nc.sync.dma_start(out=wsink_d[:], in_=wsink)

            # Running product over the 8 word-pairs: prod[p, b] accumulates
            # prod_m sigmoid(logits) for pair-slot p (word 2m+p).
            prod = prodp.tile([2, BC], F32, tag="prod", name="prod")

            def l1_matmul(acc, m, n, k):
                if m < 6:
                    lhsT = w1a_sb[:, k, ts(m, 128)]
                else:
                    lhsT = w1b_sb[:, k, ts(m - 6, 128)]
                nc.tensor.matmul(
                    acc, lhsT, xT_sb[:, n, k, :],
                    start=(k == 0), stop=(k == KT - 1),
                )

            # Epilogue as a 3-stage pipeline. Each stage's cross-engine
            # producer gets a multi-k-tile head start before the PE reaches
            # the consuming matmul, so the in-order PE never waits on ACT.
            # W2/W3 are zero-padded to 128 lhsT columns: full-width weight
            # loads are FWL-eligible and pull ahead of in-flight matmuls;
            # narrow loads serialize (~300 ns each).
            def epi_a(e):
                m, n = e["m"], e["n"]
                h1_t = h1p.tile([128, 512], BF16, tag="h1", name=f"h1_{m}_{n}")
                nc.scalar.activation(
                    h1_t, e["acc"], AF.Relu, bias=bc_sb[:, m:m + 1]
                )
                e["h1"] = h1_t

            # m0..m5 pair up for layer 3: the even m-tile's h2 lands on
            # partitions 0..63 of a shared [128, 512] tile, the odd one's
            # on 64..127, and ONE L3 matmul per pair computes 4 words'
            # logits (cols 0,1,32,33 -> readable base partitions). Halves
            # the L3 PE matmul count for those tiles (~1.3us).
            pair_h2 = {}

            def epi_b(e):
                m, n = e["m"], e["n"]
                ps2 = ps2p.tile([128, 512], F32, tag="ps2", name=f"ps2_{m}_{n}")
                nc.tensor.matmul(
                    ps2, w2_sb[:, m, :], e["h1"], start=True, stop=True
                )
                if m < 6:
                    if m % 2 == 0:
                        h2pt = h2p.tile(
                            [128, 512], BF16, tag="h2", name=f"h2pr_{m}_{n}"
                        )
                        pair_h2[(m // 2, n)] = h2pt
                        nc.scalar.activation(
                            h2pt[0:64, :], ps2[0:H1, :], AF.Relu,
                            bias=bc_sb[0:64, MT + m:MT + m + 1],
                        )
                    else:
                        h2pt = pair_h2[(m // 2, n)]
                        nc.scalar.activation(
                            h2pt[64:128, :], ps2[0:H1, :], AF.Relu,
                            bias=bc_sb[0:64, MT + m:MT + m + 1],
                        )
                        e["h2pair"] = h2pt
                else:
                    h2_t = h2p.tile([H1, 512], BF16, tag="h2", name=f"h2_{m}_{n}")
                    nc.scalar.activation(
                        h2_t, ps2[0:H1, :], AF.Relu,
                        bias=bc_sb[0:64, MT + m:MT + m + 1],
                    )
                    e["h2"] = h2_t

            sig7 = {}

            def epi_c(e):
                m, n = e["m"], e["n"]
                if m in (0, 2, 4):
                    return  # layer 3 runs at the odd pair partner
                ps3 = ps3p.tile([128, 512], F32, tag="ps3", name=f"ps3_{m}_{n}")
                if m in (1, 3, 5):
                    nc.tensor.matmul(
                        ps3, w3_sb[:, m, :], e["h2pair"], start=True, stop=True
                    )
                    # words 4p,4p+1 -> rows 0:2 (bias col = even m);
                    # words 4p+2,4p+3 -> rows 32:34 (bias col = odd m).
                    if m == 1:
                        nc.scalar.activation(
                            prod[:, ts(n, 512)], ps3[0:2, :], AF.Sigmoid,
                            bias=bc_sb[0:2, 2 * MT + m - 1:2 * MT + m],
                        )
                    else:
                        s1 = sigp.tile(
                            [2, 512], F32, tag="sig", name=f"sigL_{m}_{n}"
                        )
                        nc.scalar.activation(
                            s1, ps3[0:2, :], AF.Sigmoid,
                            bias=bc_sb[0:2, 2 * MT + m - 1:2 * MT + m],
                        )
                        nc.vector.tensor_mul(
                            prod[:, ts(n, 512)], prod[:, ts(n, 512)], s1
                        )
                    s2 = sigp.tile(
                        [2, 512], F32, tag="sig", name=f"sigH_{m}_{n}"
                    )
                    nc.scalar.activation(
                        s2, ps3[32:34, :], AF.Sigmoid,
                        bias=bc_sb[32:34, 2 * MT + m:2 * MT + m + 1],
                    )
                    nc.vector.tensor_mul(
                        prod[:, ts(n, 512)], prod[:, ts(n, 512)], s2
                    )
                    return
                nc.tensor.matmul(
                    ps3, w3_sb[0:64, m, :], e["h2"], start=True, stop=True
                )
                if m == 7:
                    # m=7's two words are packed to lhsT cols 0 and 32, so
                    # their logits land on readable base partitions 0/32 and
                    # each gets its own sigmoid (the ACT also moves the
                    # partition-32 row to a base-0 tile: DVE tensor_tensor
                    # requires equal base partitions on its SBUF inputs).
                    sa = sigp.tile([1, 512], F32, tag="sig", name=f"s7a_{n}")
                    nc.scalar.activation(
                        sa, ps3[0:1, :], AF.Sigmoid,
                        bias=bc_sb[0:1, 2 * MT + 7:2 * MT + 8],
                    )
                    sb = sigp.tile([1, 512], F32, tag="sig", name=f"s7b_{n}")
                    nc.scalar.activation(
                        sb, ps3[32:33, :], AF.Sigmoid,
                        bias=bc_sb[32:33, 2 * MT + 7:2 * MT + 8],
                    )
                    sig7[n] = (sa, sb)
                    return
                # m == 6
                sig_t = sigp.tile([2, 512], F32, tag="sig", name=f"sig_{m}_{n}")
                nc.scalar.activation(
                    sig_t, ps3[0:2, :], AF.Sigmoid,
                    bias=bc_sb[0:2, 2 * MT + m:2 * MT + m + 1],
                )
                nc.vector.tensor_mul(
                    prod[:, ts(n, 512)], prod[:, ts(n, 512)], sig_t
                )

            stage_q = []
            EPI_STAGES = (epi_a, epi_b, epi_c)

            def tick():
                # Advance the oldest pending epilogue by one stage; returns
                # the (m, n) that fully completed, if any.
                if not stage_q:
                    return None
                e = stage_q[0]
                EPI_STAGES[e["s"]](e)
                e["s"] += 1
                if e["s"] == 3:
                    stage_q.pop(0)
                    return (e["m"], e["n"])
                return None

            pre = {}

            def on_done(mn):
                # m6 completing finalizes prod for that n-half: bounce
                # partition 1 and pre-multiply, hidden under later k-loops.
                if mn in ((6, 0), (6, 1)):
                    n = mn[1]
                    r1 = prodp.tile([1, 512], F32, tag=f"r1_{n}", name=f"r1_{n}")
                    nc.sync.dma_start(out=r1, in_=prod[1:2, ts(n, 512)])
                    p = prodp.tile([1, 512], F32, tag=f"pre_{n}", name=f"pre_{n}")
                    nc.vector.tensor_mul(p, prod[0:1, ts(n, 512)], r1)
                    pre[n] = p
                elif mn == (7, 0):
                    sa, sb = sig7[0]
                    o1 = prodp.tile([1, 512], F32, tag="o1_0", name="o1_0")
                    nc.vector.tensor_mul(o1, pre[0], sa)
                    o2 = prodp.tile([1, 512], F32, tag="o2_0", name="o2_0")
                    nc.vector.tensor_mul(o2, o1, sb)
                    nc.sync.dma_start(out=out_d[:, 0:512], in_=o2)

            TICKS = tuple(range(2, 31, 2))  # 15 ticks per 32-k loop

            # Pass 1: {m0..m5} x n0, k-outer with 6 PSUM accumulators.
            # Arrival-paced: consumes xT-n0 (128 KiB/ktile) + w1a
            # (192 KiB/ktile) per 6x512-cycle matmul group = 250 GB/s.
            P1 = [(m, 0) for m in range(6)]
            accs = {
                mn: accp.tile(
                    [128, 512], F32, tag="acc", name=f"acc_{mn[0]}_{mn[1]}"
                )
                for mn in P1
            }
            for k in range(KT):
                for mn in P1:
                    l1_matmul(accs[mn], mn[0], mn[1], k)
                if k in TICKS:
                    on_done(tick())
            for mn in P1:
                stage_q.append({"m": mn[0], "n": mn[1], "acc": accs[mn], "s": 0})

            # Pass 2: {m0..m5} x n1 as six sequential single-job k-loops
            # (all data already resident - no arrival pacing needed).
            # Sequential jobs keep PSUM ring-slot reuse staggered: each
            # job's slot was freed by an epilogue pumped several k-loops
            # earlier, so no pass-boundary relu pileup stalls the PE.
            for m in range(6):
                acc = accp.tile([128, 512], F32, tag="acc", name=f"acc_{m}_1")
                for k in range(KT):
                    l1_matmul(acc, m, 1, k)
                    if k in TICKS:
                        on_done(tick())
                stage_q.append({"m": m, "n": 1, "acc": acc, "s": 0})

            # Pass 3: m6 as an n0/n1 k-interleaved pair, then m7 singles.
            acc60 = accp.tile([128, 512], F32, tag="acc", name="acc_6_0")
            acc61 = accp.tile([128, 512], F32, tag="acc", name="acc_6_1")
            for k in range(KT):
                l1_matmul(acc60, 6, 0, k)
                l1_matmul(acc61, 6, 1, k)
                if k in TICKS:
                    on_done(tick())
            stage_q.append({"m": 6, "n": 0, "acc": acc60, "s": 0})
            stage_q.append({"m": 6, "n": 1, "acc": acc61, "s": 0})

            acc70 = accp.tile([128, 512], F32, tag="acc", name="acc_7_0")
            for k in range(KT):
                l1_matmul(acc70, 7, 0, k)
                if k in TICKS:
                    on_done(tick())
            stage_q.append({"m": 7, "n": 0, "acc": acc70, "s": 0})

            # (7,1) as two independent 256-box k-loops. Half a's full
            # epilogue chain (relu->L2->relu2->L3->sigmoid->mul->mul->out
            # DMA) is pumped between half b's k-matmuls, so after the very
            # last L1 matmul only half b's chain (~3us) remains - versus
            # the whole 512-box chain (~6.5us) when (7,1) accumulated in
            # one piece. (7,0)'s stage_q epilogue drains during half a.
            def l1_matmul_h(acc, k, h):
                nc.tensor.matmul(
                    acc,
                    w1b_sb[:, k, ts(1, 128)],
                    xT_sb[:, 1, k, h * 256:(h + 1) * 256],
                    start=(k == 0),
                    stop=(k == KT - 1),
                )

            def tail_half(h, acc, stages):
                # stages: list of callables, invoked one per call index
                def relu1():
                    h1h = h1p.tile([128, 256], BF16, tag="h1", name=f"h1f_{h}")
                    nc.scalar.activation(
                        h1h, acc, AF.Relu, bias=bc_sb[:, 7:8]
                    )
                    st["h1"] = h1h

                def l2():
                    ps2h = ps2p.tile([128, 256], F32, tag="ps2", name=f"ps2f_{h}")
                    nc.tensor.matmul(
                        ps2h, w2_sb[:, 7, :], st["h1"], start=True, stop=True
                    )
                    h2h = h2p.tile([H1, 256], BF16, tag="h2", name=f"h2f_{h}")
                    nc.scalar.activation(
                        h2h, ps2h[0:H1, :], AF.Relu,
                        bias=bc_sb[0:64, MT + 7:MT + 8],
                    )
                    st["h2"] = h2h

                def l3():
                    ps3h = ps3p.tile([128, 256], F32, tag="ps3", name=f"ps3f_{h}")
                    nc.tensor.matmul(
                        ps3h, w3_sb[0:64, 7, :], st["h2"], start=True, stop=True
                    )
                    sa = sigp.tile([1, 256], F32, tag="sig", name=f"sfa_{h}")
                    nc.scalar.activation(
                        sa, ps3h[0:1, :], AF.Sigmoid,
                        bias=bc_sb[0:1, 2 * MT + 7:2 * MT + 8],
                    )
                    sb_ = sigp.tile([1, 256], F32, tag="sig", name=f"sfb_{h}")
                    nc.scalar.activation(
                        sb_, ps3h[32:33, :], AF.Sigmoid,
                        bias=bc_sb[32:33, 2 * MT + 7:2 * MT + 8],
                    )
                    st["sa"], st["sb"] = sa, sb_

                def combine():
                    sl = slice(h * 256, (h + 1) * 256)
                    o1h = prodp.tile([1, 256], F32, tag=f"o1f{h}", name=f"o1f_{h}")
                    nc.vector.tensor_mul(o1h, pre[1][:, sl], st["sa"])
                    o2h = prodp.tile([1, 256], F32, tag=f"o2f{h}", name=f"o2f_{h}")
                    nc.vector.tensor_mul(o2h, o1h, st["sb"])
                    nc.sync.dma_start(
                        out=out_d[:, 512 + h * 256:512 + (h + 1) * 256],
                        in_=o2h,
                    )

                st = {}
                stages.extend([relu1, l2, l3, combine])

            acc71a = accp.tile([128, 256], F32, tag="acc", name="acc_7_1a")
            for k in range(KT):
                l1_matmul_h(acc71a, k, 0)
                if k in TICKS:
                    on_done(tick())
            while stage_q:
                on_done(tick())

            # Half b's k-loop with half a's epilogue stages interleaved.
            a_stages = []
            tail_half(0, acc71a, a_stages)
            acc71b = accp.tile([128, 256], F32, tag="acc", name="acc_7_1b")
            A_TICKS = (2, 7, 12, 17)
            for k in range(KT):
                l1_matmul_h(acc71b, k, 1)
                if k in A_TICKS:
                    a_stages[A_TICKS.index(k)]()

            # Only half b's chain remains after the final L1 matmul.
            b_stages = []
            tail_half(1, acc71b, b_stages)
            for s in b_stages:
                s()

    nc.compile()
    return nc


_NC_CACHE = None


def _get_nc():
    global _NC_CACHE
    if _NC_CACHE is None:
        _NC_CACHE = build_nc()
    return _NC_CACHE


def _pack_inputs(x, words, W1, b1, W2, b2, W3, b3):
    bf = ml_dtypes.bfloat16
    words = np.asarray(words).astype(np.int64)

    w1g = np.asarray(W1)[words]                     # [16, 4096, 64]
    w1cat = w1g.transpose(1, 0, 2).reshape(F, NW * H1).astype(bf)  # [4096, 1024]
    # -> [p, k, col]: partition-major so each partition's whole k-range
    # is one contiguous DMA run; split m0..m5 / m6..m7 so pass 1 can
    # stream only what it consumes.
    w1p = np.ascontiguousarray(
        w1cat.reshape(KT, 128, NW * H1).transpose(1, 0, 2)
    )                                               # [128, 32, 1024]
    w1pa = np.ascontiguousarray(w1p[:, :, 0:768])   # [128, 32, 768]
    w1pb = np.ascontiguousarray(w1p[:, :, 768:1024])  # [128, 32, 256]
    b1cat = np.asarray(b1)[words].reshape(NW * H1)  # [1024]
    b1p = np.ascontiguousarray(b1cat.reshape(MT, 128).T).astype(np.float32)

    w2g = np.asarray(W2)[words]                     # [16, 64, 32]
    w2blk = np.zeros((MT, 128, 128), np.float32)
    for t in range(MT):
        w2blk[t, 0:64, 0:32] = w2g[2 * t]
        w2blk[t, 64:128, 32:64] = w2g[2 * t + 1]
    w2p = np.ascontiguousarray(w2blk.transpose(1, 0, 2)).astype(bf)  # [128,8,128]
    b2g = np.asarray(b2)[words]                     # [16, 32]
    b2blk = np.zeros((MT, 64), np.float32)
    for t in range(MT):
        b2blk[t, 0:32] = b2g[2 * t]
        b2blk[t, 32:64] = b2g[2 * t + 1]
    b2p = np.ascontiguousarray(b2blk.T).astype(np.float32)           # [64, 8]

    w3g = np.asarray(W3)[words]                     # [16, 32]
    w3blk = np.zeros((MT, 128, 128), np.float32)
    # m0..m5 pair blocks stored at the odd slot: one [128,128] lhsT per
    # pair covering 4 words (h2 rows 0:32/32:64/64:96/96:128 -> cols
    # 0/1/32/33, all readable base partitions).
    for p in range(3):
        mo = 2 * p + 1
        w3blk[mo, 0:32, 0] = w3g[4 * p]
        w3blk[mo, 32:64, 1] = w3g[4 * p + 1]
        w3blk[mo, 64:96, 32] = w3g[4 * p + 2]
        w3blk[mo, 96:128, 33] = w3g[4 * p + 3]
    # m6: block-diag cols 0/1; m7: cols 0/32 (split-sigmoid tail path).
    w3blk[6, 0:32, 0] = w3g[12]
    w3blk[6, 32:64, 1] = w3g[13]
    w3blk[7, 0:32, 0] = w3g[14]
    w3blk[7, 32:64, 32] = w3g[15]
    w3p = np.ascontiguousarray(w3blk.transpose(1, 0, 2)).astype(bf)  # [128,8,128]
    b3g = np.asarray(b3)[words]                     # [16]
    b3blk = b3g.reshape(MT, 2)
    b3p = np.zeros((34, MT), np.float32)
    b3p[0:2, :] = b3blk.T
    # pair path: rows 32:34 of odd cols hold that m-tile's two words'
    # biases; m7 keeps its odd word's bias at row 32.
    for mo in (1, 3, 5):
        b3p[32:34, mo] = b3blk[mo]
    b3p[32, 7] = b3blk[7, 1]

    # Pack b1/b2/b3 into one [128, 3*MT] f32 block (one DMA).
    bcp = np.zeros((128, 3 * MT), np.float32)
    bcp[:, 0:MT] = b1p
    bcp[0:64, MT:2 * MT] = b2p
    bcp[0:34, 2 * MT:3 * MT] = b3p

    x = np.asarray(x, dtype=np.float32)
    shared = {"w1a": w1pa, "w1b": w1pb, "w2": w2p, "w3": w3p, "bc": bcp}
    in_maps = []
    for c in range(N_CORES):
        # [p, nhalf, k, b]: partition-major, n-half major so pass 1 can
        # stream only the n0 half (one contiguous 32 KiB run per
        # partition per half).
        xc = x[c * BC:(c + 1) * BC, :].astype(bf)            # [1024, 4096]
        xT_c = np.ascontiguousarray(
            xc.T.reshape(KT, 128, 2, 512).transpose(1, 2, 0, 3)
        )                                                    # [128, 2, 32, 512]
        in_maps.append({"xT": xT_c, **shared})
    return in_maps


def _enable_trace():
    """Register the axon NTFF profile hook (the image's antenv lacks
    axon_hooks, so boot degraded silently) and disable artifact upload."""
    import sys
    import types
    import antenv
    from concourse import bass_utils as bu

    if "antenv.axon_hooks" not in sys.modules:
        mod = types.ModuleType("antenv.axon_hooks")
        mod._hook = None

        def set_axon_ntff_profile_hook(h):
            mod._hook = h

        def get_axon_ntff_profile_hook():
            return mod._hook

        mod.set_axon_ntff_profile_hook = set_axon_ntff_profile_hook
        mod.get_axon_ntff_profile_hook = get_axon_ntff_profile_hook
        sys.modules["antenv.axon_hooks"] = mod
        antenv.axon_hooks = mod

        from trn_agent_boot.trn_boot import _ntff_profile_via_ctypes

        set_axon_ntff_profile_hook(
            _ntff_profile_via_ctypes("/opt/axon/libaxon_pjrt.so")
        )

    bu.upload_artifacts = lambda tmpdir: tmpdir


def kernel(nBBox, x, words, W1, b1, W2, b2, W3, b3):
    global LAST_RESULTS
    nc = _get_nc()
    in_maps = _pack_inputs(x, words, W1, b1, W2, b2, W3, b3)
    trace = bool(int(os.environ.get("KERNEL_TRACE", "0")))
    if trace:
        _enable_trace()
    res = run_bass_kernel_spmd(
        nc, in_maps, core_ids=list(range(N_CORES)), trace=trace
    )
    LAST_RESULTS = res
    out = np.concatenate(
        [res.results[c]["out"].reshape(BC) for c in range(N_CORES)]
    )
    return out.astype(np.float32)[:, None]
